# revision 7
# baseline (speedup 1.0000x reference)
"""MultiHeadAttention Trainium2 kernel (8 NeuronCores).

Problem: B=2, C=512, S=2048, 8 heads (dh=64), 1x1-conv projections.

Sharding: core = (batch b, head-pair hp); b = core // 4, hp = core % 4.
Each core processes its batch's full q/k/v (channel rows hs=128*hp..+128 of
the projected tensors = heads 2hp, 2hp+1), runs attention for its 2 heads,
and computes the partial output projection Wo[:, hs:hs+128] @ attn_pair
-> (512, 2048) partial.  Host sums the 4 partials per batch (+ bo).

On-core dataflow (all matmuls bf16 in / fp32 psum out):
  kp/qp   (128=2*dh ch, 2048 s)  = WT_slice.T @ x  (+bias via K=1 matmul)
  vpT_h   (2048 s, 128) sbuf bf16; per 128-k-tile layout [v(64) | ones(64)]
  scores  psum (128 k, 1024 = h0 512 q | h1 512 q), 2 heads via PE row-tiles
  expS    = ACT Exp(scale=1/8) -> bf16 (no max subtraction: |scores*scale|<~6)
  AV      psum (128, 512) per head: rows 0-63 = unnormalized attn,
          rows 64-127 = softmax denominator replicated (ones cols of vpT)
  attnC   = AV[0:64] * reciprocal_approx_fast(AV[64:128])  -> bf16
  out     partial = woT_slice.T @ attnC -> fp32 -> DRAM
"""

import numpy as np
import ml_dtypes
from contextlib import ExitStack

import concourse.bass as bass
from concourse import bacc
import concourse.mybir as mybir
import concourse.tile as tile
from concourse.bass_utils import run_bass_kernel_spmd

B = 2
C = 512
S = 2048
NH = 8
DH = C // NH            # 64
HP = 128                # channel rows per core (2 heads)
N_CORES = 8
SC = 512                # q-chunk / s-chunk size
NSC = S // SC           # 4
NKT = S // 128          # 16 k-tiles
SCALE = float(DH) ** -0.5  # 0.125

BF = mybir.dt.bfloat16
F32 = mybir.dt.float32
EXP = mybir.ActivationFunctionType.Exp
BF_NP = ml_dtypes.bfloat16

_NC_CACHE = None


def build_nc():
    nc = bacc.Bacc(None)

    xq = nc.declare_dram_parameter("xq", [C, S], BF, isOutput=False)
    xk = nc.declare_dram_parameter("xk", [C, S], BF, isOutput=False)
    xv = nc.declare_dram_parameter("xv", [C, S], BF, isOutput=False)
    wqT = nc.declare_dram_parameter("wqT", [C, HP], BF, isOutput=False)
    wkT = nc.declare_dram_parameter("wkT", [C, HP], BF, isOutput=False)
    wvT = nc.declare_dram_parameter("wvT", [C, HP], BF, isOutput=False)
    woT = nc.declare_dram_parameter("woT", [HP, C], BF, isOutput=False)
    bq_r = nc.declare_dram_parameter("bq_r", [1, HP], BF, isOutput=False)
    bk_r = nc.declare_dram_parameter("bk_r", [1, HP], BF, isOutput=False)
    bv_r = nc.declare_dram_parameter("bv_r", [1, HP], BF, isOutput=False)
    out_d = nc.declare_dram_parameter("out", [C, S], F32, isOutput=True)

    with tile.TileContext(nc) as tc:
        with ExitStack() as ctx:
            consts = ctx.enter_context(tc.tile_pool(name="consts", bufs=1))
            xs = ctx.enter_context(tc.tile_pool(name="xs", bufs=12))
            proj = ctx.enter_context(tc.tile_pool(name="proj", bufs=1))
            epool = ctx.enter_context(tc.tile_pool(name="epool", bufs=3))
            apool = ctx.enter_context(tc.tile_pool(name="apool", bufs=2))
            rpool = ctx.enter_context(tc.tile_pool(name="rpool", bufs=2))
            opool = ctx.enter_context(tc.tile_pool(name="opool", bufs=3))
            ps_s = ctx.enter_context(tc.tile_pool(name="ps_s", bufs=2, space="PSUM"))
            ps_av = ctx.enter_context(tc.tile_pool(name="ps_av", bufs=1, space="PSUM"))
            ps_sm = ctx.enter_context(tc.tile_pool(name="ps_sm", bufs=2, space="PSUM"))

            # Warm the ACT exp table early so the ~2.7us table load overlaps
            # the input DMA phase.
            warm = consts.tile([1, 8], F32, tag="warm")
            nc.vector.memset(warm, 0.0)
            nc.scalar.activation(out=warm, in_=warm, func=EXP)

            # Weights / constant rows.
            wq_sb = consts.tile([128, 4, HP], BF, tag="wq")
            wk_sb = consts.tile([128, 4, HP], BF, tag="wk")
            wv_sb = consts.tile([128, 4, HP], BF, tag="wv")
            wo_sb = consts.tile([HP, C], BF, tag="wo")
            ones_sb = consts.tile([1, SC], BF, tag="ones")
            bq_sb = consts.tile([1, HP], BF, tag="bq")
            bk_sb = consts.tile([1, HP], BF, tag="bk")
            bv_sb = consts.tile([1, HP], BF, tag="bv")
            nc.gpsimd.dma_start(out=wk_sb, in_=wkT[:, :].rearrange("(t p) d -> p t d", p=128))
            nc.gpsimd.dma_start(out=wq_sb, in_=wqT[:, :].rearrange("(t p) d -> p t d", p=128))
            nc.gpsimd.dma_start(out=wv_sb, in_=wvT[:, :].rearrange("(t p) d -> p t d", p=128))
            nc.sync.dma_start(out=wo_sb, in_=woT[:, :])
            nc.vector.memset(ones_sb, 1.0)
            nc.sync.dma_start(out=bq_sb, in_=bq_r[:, :])
            nc.sync.dma_start(out=bk_sb, in_=bk_r[:, :])
            nc.sync.dma_start(out=bv_sb, in_=bv_r[:, :])

            # Persistent projected tensors.
            kp_sb = proj.tile([128, S], BF, tag="kp")
            qp_sb = proj.tile([128, S], BF, tag="qp")
            vh_sb = [proj.tile([128, S // 2], BF, tag=f"vh{h}", name=f"vh{h}") for h in range(2)]
            ones64 = consts.tile([128, 64], BF, tag="ones64")
            nc.vector.memset(ones64, 1.0)

            def project_kq(x_dram, w_sb, b_sb, dst, chunks):
                for sc in chunks:
                    xt = xs.tile([128, 4, SC], BF, tag="xt")
                    nc.gpsimd.dma_start(
                        out=xt,
                        in_=x_dram[:, :].rearrange("(t p) s -> p t s", p=128)[
                            :, :, SC * sc : SC * (sc + 1)
                        ],
                    )
                    ps = ps_sm.tile([128, SC], F32, tag="sm")
                    for ci in range(4):
                        nc.tensor.matmul(
                            out=ps, lhsT=w_sb[:, ci, :], rhs=xt[:, ci, :],
                            start=(ci == 0), stop=False,
                        )
                    nc.tensor.matmul(
                        out=ps, lhsT=b_sb, rhs=ones_sb, start=False, stop=True,
                    )
                    nc.vector.tensor_copy(out=dst[:, SC * sc : SC * (sc + 1)], in_=ps)

            # k first (needed by all score tiles of q-chunk 0), then q-chunk 0.
            project_kq(xk, wk_sb, bk_sb, kp_sb, range(NSC))
            project_kq(xq, wq_sb, bq_sb, qp_sb, [0])

            # vpT: per s-tile T: psum (128 s, 128 ch) = x_v_tile.T @ WvT (+ bv)
            for sc in range(NSC):
                xt = xs.tile([128, 4, SC], BF, tag="xt")
                nc.sync.dma_start(
                    out=xt,
                    in_=xv[:, :].rearrange("(t p) s -> p t s", p=128)[
                        :, :, SC * sc : SC * (sc + 1)
                    ],
                )
                for j in range(4):
                    T = 4 * sc + j
                    ps = ps_sm.tile([128, SC], F32, tag="sm")
                    psv = ps[:, 0:128]
                    for ci in range(4):
                        nc.tensor.matmul(
                            out=psv, lhsT=xt[:, ci, 128 * j : 128 * (j + 1)],
                            rhs=wv_sb[:, ci, :], start=(ci == 0), stop=False,
                        )
                    nc.tensor.matmul(
                        out=psv, lhsT=ones_sb[:, 0:128], rhs=bv_sb,
                        start=False, stop=True,
                    )
                    nc.vector.tensor_copy(
                        out=vh_sb[0][:, 64 * T : 64 * T + 64], in_=psv[:, 0:64]
                    )
                    nc.vector.tensor_copy(
                        out=vh_sb[1][:, 64 * T : 64 * T + 64], in_=psv[:, 64:128]
                    )

            project_kq(xq, wq_sb, bq_sb, qp_sb, range(1, NSC))

            # Attention + output projection, per q-chunk.
            for qc in range(NSC):
                qsl = slice(SC * qc, SC * (qc + 1))
                avn = ps_av.tile([128, SC], F32, tag="avn")
                avs = ps_av.tile([128, SC], F32, tag="avs")
                for t in range(NKT):
                    ksl = slice(128 * t, 128 * (t + 1))
                    st = ps_s.tile([128, 2 * SC], F32, tag="sc")
                    nc.tensor.matmul(
                        out=st[:, 0:SC], lhsT=kp_sb[0:64, ksl], rhs=qp_sb[0:64, qsl],
                        start=True, stop=True, tile_position=(0, 0),
                    )
                    nc.tensor.matmul(
                        out=st[:, SC : 2 * SC], lhsT=kp_sb[64:128, ksl],
                        rhs=qp_sb[64:128, qsl],
                        start=True, stop=True, tile_position=(64, 0),
                    )
                    ex = epool.tile([128, 2 * SC], BF, tag="ex")
                    nc.scalar.activation(out=ex, in_=st, func=EXP, scale=SCALE)
                    vsl = slice(64 * t, 64 * (t + 1))
                    for h in range(2):
                        hp_sl = slice(64 * h, 64 * (h + 1))
                        exh = ex[:, SC * h : SC * (h + 1)]
                        nc.tensor.matmul(
                            out=avn[hp_sl, :], lhsT=vh_sb[h][:, vsl], rhs=exh,
                            start=(t == 0), stop=(t == NKT - 1),
                            tile_position=(0, 64 * h), skip_group_check=True,
                        )
                        nc.tensor.matmul(
                            out=avs[hp_sl, :], lhsT=ones64, rhs=exh,
                            start=(t == 0), stop=(t == NKT - 1),
                            tile_position=(0, 64 * h), skip_group_check=True,
                        )
                atile = apool.tile([128, SC], BF, tag="at")
                rb = rpool.tile([128, SC], F32, tag="rb")
                nc.vector.reciprocal_approx_fast(out=rb, in_=avs)
                nc.vector.tensor_mul(out=atile, in0=avn, in1=rb)
                for ot in range(4):
                    ps = ps_sm.tile([128, SC], F32, tag="sm")
                    nc.tensor.matmul(
                        out=ps, lhsT=wo_sb[:, 128 * ot : 128 * (ot + 1)], rhs=atile,
                        start=True, stop=True,
                    )
                    osb = opool.tile([128, SC], F32, tag="ob")
                    nc.vector.tensor_copy(out=osb, in_=ps)
                    nc.sync.dma_start(
                        out=out_d[128 * ot : 128 * (ot + 1), qsl], in_=osb
                    )

    nc.compile()
    return nc


def make_in_maps(q, k, v, Wq, bq, Wk, bk, Wv, bv, Wo, bo):
    q, k, v = (np.asarray(t, np.float32) for t in (q, k, v))
    Wq, Wk, Wv, Wo = (np.asarray(t, np.float32) for t in (Wq, Wk, Wv, Wo))
    bq, bk, bv = (np.asarray(t, np.float32) for t in (bq, bk, bv))
    in_maps = []
    for core in range(N_CORES):
        b, hp = core // 4, core % 4
        hs = slice(HP * hp, HP * (hp + 1))
        in_maps.append({
            "xq": np.ascontiguousarray(q[b, :, 0, :]).astype(BF_NP),
            "xk": np.ascontiguousarray(k[b, :, 0, :]).astype(BF_NP),
            "xv": np.ascontiguousarray(v[b, :, 0, :]).astype(BF_NP),
            "wqT": np.ascontiguousarray(Wq[hs, :].T).astype(BF_NP),
            "wkT": np.ascontiguousarray(Wk[hs, :].T).astype(BF_NP),
            "wvT": np.ascontiguousarray(Wv[hs, :].T).astype(BF_NP),
            "woT": np.ascontiguousarray(Wo[:, hs].T).astype(BF_NP),
            "bq_r": bq[hs].reshape(1, HP).astype(BF_NP),
            "bk_r": bk[hs].reshape(1, HP).astype(BF_NP),
            "bv_r": bv[hs].reshape(1, HP).astype(BF_NP),
        })
    return in_maps


def assemble_output(results, bo):
    bo = np.asarray(bo, np.float32)
    out = np.zeros((B, C, 1, S), np.float32)
    for b in range(B):
        acc = np.zeros((C, S), np.float32)
        for hp in range(4):
            acc += np.asarray(results[b * 4 + hp]["out"], np.float32)
        out[b, :, 0, :] = acc + bo[:, None]
    return out


def kernel(q, k, v, Wq, bq, Wk, bk, Wv, bv, Wo, bo):
    global _NC_CACHE
    if _NC_CACHE is None:
        _NC_CACHE = build_nc()
    nc = _NC_CACHE
    in_maps = make_in_maps(q, k, v, Wq, bq, Wk, bk, Wv, bv, Wo, bo)
    res = run_bass_kernel_spmd(nc, in_maps, list(range(N_CORES)))
    return assemble_output(res.results, bo)
